# revision 13
# baseline (speedup 1.0000x reference)
"""DeepGAT (4-layer GAT + BN + residual + MLP head) on 8 Trainium2 cores.

Sharding: nodes are dst-partitioned across the 8 cores (1250 nodes/core).
Edges are routed on the host to the core owning their dst node and sorted by
dst. Weights are replicated.

Key idea vs the straightforward scheme: aggregate in INPUT space. Since
xl = h @ W is linear, sum_e att_e * xl[src_e] = (sum_e att_e * h[src_e]) @ W,
so each edge only needs the 128-dim h row (bf16, 512B padded) instead of the
1024-dim projected row. Per layer each core:
  1. transposes its own h block-wise to node-major, appends alpha_src
     (h @ ws), writes [h | a_src | pad] rows to a bounce buffer,
  2. AllGathers the bounce -> full node-major gather table [10240, 256] bf16,
  3. per dst block: dma_gathers src rows, computes pe = exp(lrelu(as+ad))
     with alpha_dst expanded edge-wise by a one-hot matmul (no second
     gather), aggregates agg[dst] += pe * h_src and den[dst] += pe via
     one-hot matmuls into PSUM,
  4. epilogue: normalizes by den per (dst, head), transposes per head and
     projects with W_head (accumulating the head mean), then affine (BN) +
     residual + elu.
The one-hot matrices oh [e,d] / ohT [d,e] are layer-invariant and stay
resident in SBUF. Softmax normalization is applied AFTER aggregation
(mathematically identical, single edge pass).
"""

import numpy as np

import concourse.bass as bass
import concourse.bacc as bacc
import concourse.mybir as mybir
from concourse.tile import TileContext
from concourse.tile_rust import add_dep_helper

FP32 = mybir.dt.float32
BF16 = mybir.dt.bfloat16
I16 = mybir.dt.int16
AF = mybir.ActivationFunctionType
OP = mybir.AluOpType

# problem constants (hardcoded per harness contract)
ALPHA = 0.1
BN_EPS = 1e-5
NEG_SLOPE = 0.2
NEG_BIG = -30000.0
HID = 128  # partition width; fixed
ROWE = 256  # gather row elements (bf16): [h(128) | a_src(8) | pad] = 512B
SPLIT_CHUNKS = 6  # target chunks per dma_gather


def _set_dims(n=10000, e=160000, in_dim=512, heads=8, layers=4, cls=2, cores=8):
    """Set problem dims as module globals (parametrized for sim tests)."""
    g = globals()
    g["N"], g["E"], g["IN"], g["H"], g["L"], g["CLS"], g["M"] = (
        n, e, in_dim, heads, layers, cls, cores)
    g["NPC"] = n // cores
    g["NPC_PAD"] = -(-g["NPC"] // 128) * 128
    g["NBLK"] = g["NPC_PAD"] // 128
    g["NROWS"] = cores * g["NPC_PAD"]


_set_dims()


class Cfg:
    """Static schedule computed from the actual edge data."""

    def __init__(self, chunks_per_block):
        self.chunks_per_block = list(chunks_per_block)
        self.CH = sum(self.chunks_per_block)
        self.TOTE = 128 * self.CH
        self.CHP = -(-self.CH // 128) * 128


def _pack_idx16(idx, pad_to=None):
    """Pack int16 indices for dma_gather: idx i at [i%16, i//16], replicated
    to 128 partitions."""
    idx = np.asarray(idx, np.int64)
    n = len(idx)
    if pad_to is not None:
        assert pad_to >= n
        idx = np.concatenate([idx, np.zeros(pad_to - n, np.int64)])
        n = pad_to
    assert n % 16 == 0
    a = idx.astype(np.int16).reshape(n // 16, 16).T  # [16, n//16]
    return np.tile(a, (8, 1)).copy()  # [128, n//16]


def preprocess(x, edge_index, Wp, bp, Wl, att_src, att_dst, bl, gamma, beta,
               W1, b1, W2, b2):
    """Host-side: edge routing/sorting per core + weight folding."""
    x = np.asarray(x, np.float32)
    src = np.concatenate([np.asarray(edge_index[0]), np.arange(N)]).astype(np.int64)
    dst = np.concatenate([np.asarray(edge_index[1]), np.arange(N)]).astype(np.int64)

    per_core = []
    for k in range(M):
        m = (dst // NPC) == k
        s_k, d_k = src[m], dst[m] - k * NPC
        order = np.argsort(d_k, kind="stable")
        per_core.append((s_k[order], d_k[order]))

    counts = np.zeros((M, NBLK), np.int64)
    for k in range(M):
        _, d_k = per_core[k]
        b = d_k // 128
        for bb in range(NBLK):
            counts[k, bb] = int((b == bb).sum())
    chunks_per_block = [max(1, int(np.ceil(counts[:, bb].max() / 128)))
                        for bb in range(NBLK)]
    cfg = Cfg(chunks_per_block)

    per_core_inputs = []
    for k in range(M):
        s_k, d_k = per_core[k]
        b_k = d_k // 128
        srcrow = np.zeros(cfg.TOTE, np.int64)
        dstloc = np.zeros(cfg.TOTE, np.int64)
        padb = np.full(cfg.TOTE, NEG_BIG, np.float32)
        off = 0
        for bb in range(NBLK):
            sel = b_k == bb
            cnt = int(sel.sum())
            cap = 128 * cfg.chunks_per_block[bb]
            assert cnt <= cap, (k, bb, cnt, cap)
            s_sel = s_k[sel]
            # global node id -> padded table row id
            srcrow[off:off + cnt] = (s_sel // NPC) * NPC_PAD + (s_sel % NPC)
            dstloc[off:off + cnt] = d_k[sel] - 128 * bb
            padb[off:off + cnt] = 0.0
            off += cap
        assert off == cfg.TOTE

        dloc_cols = dstloc.reshape(cfg.CH, 128)  # [CH, 128]

        xT_own = np.zeros((IN, NPC_PAD), np.float32)
        xT_own[:, :NPC] = x[k * NPC:(k + 1) * NPC].T

        per_core_inputs.append({
            "srcidx": _pack_idx16(srcrow),
            "dstloc": dloc_cols.T.astype(np.float32).copy(),
            "padbias": padb.reshape(cfg.CH, 128).T.copy(),
            "xT_own": xT_own,
        })

    # weight folding
    Wl = np.asarray(Wl, np.float32)          # [L, HID, H*HID]
    a_s = np.asarray(att_src, np.float32)    # [L, H, HID]
    a_d = np.asarray(att_dst, np.float32)
    wsd = np.zeros((HID, L * 2 * H), np.float32)
    for i in range(L):
        w3 = Wl[i].reshape(HID, H, HID)
        wsd[:, i * 2 * H:i * 2 * H + H] = np.einsum("khc,hc->kh", w3, a_s[i])
        wsd[:, i * 2 * H + H:(i + 1) * 2 * H] = np.einsum("khc,hc->kh", w3, a_d[i])

    bn_inv = 1.0 / np.sqrt(1.0 + BN_EPS)
    gamma = np.asarray(gamma, np.float32)
    beta = np.asarray(beta, np.float32)
    bl = np.asarray(bl, np.float32)
    # h = elu((1-a)*(gamma*bn_inv*(mean+bl)+beta) + a*prev); fold 1/H into s.
    s_aff = ((1.0 - ALPHA) * gamma * bn_inv / H).T.copy()            # [HID, L]
    t_aff = ((1.0 - ALPHA) * (gamma * bn_inv * bl + beta)).T.copy()  # [HID, L]

    iota_sq = np.broadcast_to(np.arange(128, dtype=np.float32), (128, 128)).copy()
    ident = np.eye(128, dtype=np.float32)

    shared = {
        "Wp": np.asarray(Wp, np.float32),
        "bp": np.asarray(bp, np.float32)[:, None],
        "Wl3": Wl,  # [L, HID, H*HID] fp32, bf16-truncated on device
        "wsd": wsd,
        "s_aff": s_aff, "t_aff": t_aff,
        "W1": np.asarray(W1, np.float32),
        "b1": np.asarray(b1, np.float32)[:, None],
        "W2": np.asarray(W2, np.float32),
        "b2": np.asarray(b2, np.float32)[:, None],
        "iota_sq": iota_sq, "ident": ident,
    }
    return cfg, shared, per_core_inputs


def _elu(nc, p, out_ap, z_ap, shape, tg):
    """out = elu(z) = relu(z) + exp(min(z,0)) - 1, z in SBUF f32."""
    P, F = shape
    mn = p.tile([P, F], FP32, tag=f"elu_mn_{tg}")
    ex = p.tile([P, F], FP32, tag=f"elu_ex_{tg}")
    rl = p.tile([P, F], FP32, tag=f"elu_rl_{tg}")
    nc.vector.tensor_scalar_min(out=mn[:], in0=z_ap, scalar1=0.0)
    nc.scalar.activation(out=ex[:], in_=mn[:], func=AF.Exp)
    nc.vector.tensor_scalar_max(out=rl[:], in0=z_ap, scalar1=0.0)
    nc.vector.tensor_tensor(out=rl[:], in0=rl[:], in1=ex[:], op=OP.add)
    nc.vector.tensor_scalar_sub(out=out_ap, in0=rl[:], scalar1=1.0)


def build(nc, cfg):
    """Emit the SPMD program (dims from module globals)."""
    n, npc, npc_pad = N, NPC, NPC_PAD
    in_dim, layers, heads, cores = IN, L, H, M
    nblk = NBLK
    qd = HID // 2
    CH, CHP = cfg.CH, cfg.CHP

    # ---------------- I/O ----------------
    srcidx = nc.dram_tensor("srcidx", [128, cfg.TOTE // 16], I16, kind="ExternalInput")
    dstloc_in = nc.dram_tensor("dstloc", [128, CH], FP32, kind="ExternalInput")
    padbias_in = nc.dram_tensor("padbias", [128, CH], FP32, kind="ExternalInput")
    xT_own_in = nc.dram_tensor("xT_own", [in_dim, npc_pad], FP32, kind="ExternalInput")
    Wp_in = nc.dram_tensor("Wp", [in_dim, HID], FP32, kind="ExternalInput")
    bp_in = nc.dram_tensor("bp", [HID, 1], FP32, kind="ExternalInput")
    Wl3_in = nc.dram_tensor("Wl3", [layers, HID, heads * HID], FP32,
                            kind="ExternalInput")
    wsd_in = nc.dram_tensor("wsd", [HID, layers * 2 * heads], FP32,
                            kind="ExternalInput")
    s_aff_in = nc.dram_tensor("s_aff", [HID, layers], FP32, kind="ExternalInput")
    t_aff_in = nc.dram_tensor("t_aff", [HID, layers], FP32, kind="ExternalInput")
    W1_in = nc.dram_tensor("W1", [HID, qd], FP32, kind="ExternalInput")
    b1_in = nc.dram_tensor("b1", [qd, 1], FP32, kind="ExternalInput")
    W2_in = nc.dram_tensor("W2", [qd, CLS], FP32, kind="ExternalInput")
    b2_in = nc.dram_tensor("b2", [CLS, 1], FP32, kind="ExternalInput")
    iota_in = nc.dram_tensor("iota_sq", [128, 128], FP32, kind="ExternalInput")
    ident_in = nc.dram_tensor("ident", [128, 128], FP32, kind="ExternalInput")
    out_dram = nc.dram_tensor("out", [CLS, npc_pad], FP32, kind="ExternalOutput")

    table = nc.dram_tensor("h_table", [NROWS, ROWE], BF16,
                           addr_space="Shared" if cores > 4 else "Local")

    with TileContext(nc) as tc:
        with (
            tc.tile_pool(name="const", bufs=1) as cpool,
            tc.tile_pool(name="hbuf", bufs=1) as hpool,
            tc.tile_pool(name="row", bufs=2) as rpool,
            tc.tile_pool(name="gath", bufs=7) as gpool,
            tc.tile_pool(name="edge", bufs=4) as epool,
            tc.tile_pool(name="blk", bufs=2) as bpool,
            tc.tile_pool(name="wide", bufs=1) as wpool,
            tc.tile_pool(name="dram", bufs=1, space="DRAM") as dpool,
            tc.tile_pool(name="psA", bufs=2, space="PSUM") as psA,
            tc.tile_pool(name="psD", bufs=1, space="PSUM") as psD,
            tc.tile_pool(name="psE", bufs=1, space="PSUM") as psE,
            tc.tile_pool(name="psT", bufs=2, space="PSUM") as psT,
        ):
            # dma_gather allocates a register per distinct count; cache them
            _regs = {}

            def nreg(v):
                if v not in _regs:
                    _regs[v] = nc.gpsimd.to_reg(v)
                return _regs[v]

            # ---------------- resident constants / state ----------------
            iota_bf = cpool.tile([128, 128], BF16)
            nc.gpsimd.dma_start(out=iota_bf[:], in_=iota_in[:, :])
            ident_f = cpool.tile([128, 128], FP32)
            nc.sync.dma_start(out=ident_f[:], in_=ident_in[:, :])
            ident_bf = cpool.tile([128, 128], BF16)
            nc.gpsimd.dma_start(out=ident_bf[:], in_=ident_in[:, :])
            dstloc_f = cpool.tile([128, CH], FP32)
            nc.sync.dma_start(out=dstloc_f[:], in_=dstloc_in[:, :])
            srcidx_sb = cpool.tile([128, cfg.TOTE // 16], I16)
            nc.sync.dma_start(out=srcidx_sb[:], in_=srcidx[:, :])
            padbias = cpool.tile([128, CH], FP32)
            nc.sync.dma_start(out=padbias[:], in_=padbias_in[:, :])
            wsd_sb = cpool.tile([128, layers * 2 * heads], FP32)
            nc.sync.dma_start(out=wsd_sb[:], in_=wsd_in[:, :])
            Wl_bf = cpool.tile([128, layers, heads * HID], BF16)
            for li in range(layers):
                nc.gpsimd.dma_start(out=Wl_bf[:, li, :], in_=Wl3_in[li, :, :])
            s_aff = cpool.tile([128, layers], FP32)
            nc.sync.dma_start(out=s_aff[:], in_=s_aff_in[:, :])
            t_aff = cpool.tile([128, layers], FP32)
            nc.sync.dma_start(out=t_aff[:], in_=t_aff_in[:, :])
            W1_sb = cpool.tile([128, qd], FP32)
            nc.sync.dma_start(out=W1_sb[:], in_=W1_in[:, :])
            b1_sb = cpool.tile([qd, 1], FP32)
            nc.sync.dma_start(out=b1_sb[:], in_=b1_in[:, :])
            W2_sb = cpool.tile([qd, CLS], FP32)
            nc.sync.dma_start(out=W2_sb[:], in_=W2_in[:, :])
            b2_sb = cpool.tile([CLS, 1], FP32)
            nc.sync.dma_start(out=b2_sb[:], in_=b2_in[:, :])
            bp_sb = cpool.tile([HID, 1], FP32)
            nc.sync.dma_start(out=bp_sb[:], in_=bp_in[:, :])

            h_own = [hpool.tile([128, npc_pad], FP32, tag=f"h_own{i}",
                                name=f"h_own{i}")
                     for i in range(2)]
            ad_all = hpool.tile([128, nblk * heads], BF16, tag="ad_all")

            # one-hot matrices, layer-invariant, SBUF-resident
            oh_all = cpool.tile([128, CH, 128], BF16)
            ohT_all = cpool.tile([128, CH, 128], BF16)
            for c in range(CH):
                nc.vector.tensor_scalar(
                    out=oh_all[:, c, :], in0=iota_bf[:],
                    scalar1=dstloc_f[:, c:c + 1], scalar2=None,
                    op0=OP.is_equal)
                ohT_ps = psT.tile([128, 128], BF16, tag="tr", name=f"ohT{c}")
                nc.tensor.transpose(out=ohT_ps[:], in_=oh_all[:, c, :],
                                    identity=ident_bf[:])
                nc.scalar.activation(out=ohT_all[:, c, :], in_=ohT_ps[:],
                                     func=AF.Copy)

            kchunks = in_dim // 128

            # ------- h0 = elu(x @ Wp + bp), own nodes only ----
            with tc.tile_pool(name="x0", bufs=2) as x0pool:
                Wp_sb = cpool.tile([128, kchunks, HID], FP32)
                for kc in range(kchunks):
                    nc.sync.dma_start(out=Wp_sb[:, kc, :],
                                      in_=Wp_in[kc * 128:(kc + 1) * 128, :])
                z0 = wpool.tile([128, npc_pad], FP32, tag="zw")
                h0a = psA.tile([128, 1024], FP32, tag="agg", name="h0a")
                h0b = psD.tile([128, 256], FP32, tag="den", name="h0b")
                pieces0 = [(0, 512, h0a[:, 0:512]), (512, 1024, h0a[:, 512:1024]),
                           (1024, npc_pad, h0b[:, 0:npc_pad - 1024])]
                for kc in range(kchunks):
                    xt = x0pool.tile([128, npc_pad], FP32, tag="xT",
                                     name=f"xT{kc}")
                    nc.sync.dma_start(out=xt[:],
                                      in_=xT_own_in[kc * 128:(kc + 1) * 128, :])
                    for (j0, j1, ps) in pieces0:
                        nc.tensor.matmul(out=ps,
                                         lhsT=Wp_sb[:, kc, :],
                                         rhs=xt[:, j0:j1],
                                         start=(kc == 0),
                                         stop=(kc == kchunks - 1),
                                         skip_group_check=True)
                for (j0, j1, ps) in pieces0:
                    nc.scalar.activation(out=z0[:, j0:j1], in_=ps,
                                         func=AF.Identity,
                                         bias=bp_sb[:, :1], scale=1.0)
                _elu(nc, wpool, h_own[0][:], z0[:], (128, npc_pad), "w")

            # ---------------- layers ----------------
            for li in range(layers):
                hprev = h_own[li % 2]
                hnew = h_own[(li + 1) % 2]

                # --- own-table build: node-major [h | a_src] rows ---
                bounce = dpool.tile([npc_pad, ROWE], BF16, tag="bounce")
                for bb in range(nblk):
                    hb_bf = rpool.tile([128, 128], BF16, tag="hbf")
                    nc.vector.tensor_copy(
                        out=hb_bf[:], in_=hprev[:, bb * 128:(bb + 1) * 128])
                    al_ps = psE.tile([128, 2 * heads], FP32, tag="ade",
                                     name=f"al{li}_{bb}")
                    nc.tensor.matmul(out=al_ps[:],
                                     lhsT=hprev[:, bb * 128:(bb + 1) * 128],
                                     rhs=wsd_sb[:, li * 2 * heads:
                                                (li + 1) * 2 * heads],
                                     start=True, stop=True,
                                     skip_group_check=True)
                    t_ps = psT.tile([128, 128], BF16, tag="tr",
                                    name=f"tps{li}_{bb}")
                    nc.tensor.transpose(out=t_ps[:], in_=hb_bf[:],
                                        identity=ident_bf[:])
                    row_sb = rpool.tile([128, ROWE], BF16, tag="row")
                    nc.scalar.activation(out=row_sb[:, 0:128], in_=t_ps[:],
                                         func=AF.Copy)
                    nc.vector.tensor_copy(out=row_sb[:, 128:128 + heads],
                                          in_=al_ps[:, 0:heads])
                    nc.vector.tensor_copy(
                        out=ad_all[:, bb * heads:(bb + 1) * heads],
                        in_=al_ps[:, heads:2 * heads])
                    nc.sync.dma_start(
                        out=bounce[bb * 128:(bb + 1) * 128, :], in_=row_sb[:])

                # --- AllGather: bounce -> full gather table ---
                cc = nc.gpsimd.collective_compute(
                    "AllGather", OP.bypass,
                    replica_groups=[list(range(cores))],
                    ins=[bounce[:]], outs=[table[:, :]],
                )

                # --- edge phase: per-split batched attention, pipelined ---
                smetas = []
                off = 0
                for bb in range(nblk):
                    cb = cfg.chunks_per_block[bb]
                    nsp = max(1, -(-cb // SPLIT_CHUNKS))
                    base = cb // nsp
                    rem = cb % nsp
                    sizes = [base + (1 if i < rem else 0) for i in range(nsp)]
                    lo = 0
                    for sz in sizes:
                        g = gpool.tile([128, sz, ROWE], BF16, tag="gt")
                        g_ = nc.gpsimd.dma_gather(
                            out_ap=g[:], in_ap=table[:, :],
                            idxs_ap=srcidx_sb[:, (off + lo) * 8:
                                              (off + lo + sz) * 8],
                            num_idxs=128 * sz, num_idxs_reg=nreg(128 * sz),
                            elem_size=ROWE, single_packet=128 * sz <= 1024)
                        add_dep_helper(g_.ins, cc.ins, True, "tbl->gather")
                        smetas.append((off + lo, bb, g, sz, lo, cb))
                        lo += sz
                    off += cb

                state = {}
                pend = {}

                def stageA(s):
                    c0, bb, gt, sz, lo, cb = smetas[s]
                    ade6 = psE.tile([128, sz, heads], FP32, tag="ade",
                                    name=f"ade{li}_{c0}")
                    for j in range(sz):
                        nc.tensor.matmul(
                            out=ade6[:, j, :], lhsT=ohT_all[:, c0 + j, :],
                            rhs=ad_all[:, bb * heads:(bb + 1) * heads],
                            start=True, stop=True, skip_group_check=True)
                    # sv = a_src[gathered] + a_dst[one-hot] + padbias
                    sv6 = epool.tile([128, sz, heads], FP32, tag="sv")
                    nc.vector.tensor_tensor(
                        out=sv6[:], in0=gt[:, :, 128:128 + heads],
                        in1=ade6[:], op=OP.add)
                    nc.vector.tensor_tensor(
                        out=sv6[:], in0=sv6[:],
                        in1=padbias[:, c0:c0 + sz, None].to_broadcast(
                            [128, sz, heads]),
                        op=OP.add)
                    # exp(lrelu(x)) = max(exp(x), exp(0.2*x)), exp monotone;
                    # pads carry -3e4 so both exps are 0 there
                    e1 = epool.tile([128, sz, heads], FP32, tag="e1")
                    nc.scalar.activation(
                        out=e1[:].rearrange("p a b -> p (a b)"),
                        in_=sv6[:].rearrange("p a b -> p (a b)"), func=AF.Exp)
                    e2 = epool.tile([128, sz, heads], FP32, tag="e2")
                    nc.scalar.activation(
                        out=e2[:].rearrange("p a b -> p (a b)"),
                        in_=sv6[:].rearrange("p a b -> p (a b)"), func=AF.Exp,
                        scale=NEG_SLOPE)
                    pe6 = epool.tile([128, sz, heads], BF16, tag="pe6")
                    nc.vector.tensor_tensor(out=pe6[:], in0=e1[:], in1=e2[:],
                                            op=OP.max)
                    pend[s] = pe6

                def stageB(s):
                    c0, bb, gt, sz, lo, cb = smetas[s]
                    pe6 = pend.pop(s)
                    for j in range(sz):
                        c = c0 + j
                        first, last = lo + j == 0, lo + j == cb - 1
                        if first:
                            state[bb] = (
                                psA.tile([128, heads * HID], FP32, tag="agg",
                                         name=f"agg{li}_{bb}"),
                                psD.tile([128, heads], FP32, tag="den",
                                         name=f"den{li}_{bb}"))
                        agg, den = state[bb]
                        nc.tensor.matmul(
                            out=den[:], lhsT=oh_all[:, c, :],
                            rhs=pe6[:, j, :],
                            start=first, stop=last, skip_group_check=True)
                        msg = epool.tile([128, heads, HID], BF16, tag="msg")
                        nc.vector.tensor_tensor(
                            out=msg[:],
                            in0=gt[:, j:j + 1, 0:HID].to_broadcast(
                                [128, heads, HID]),
                            in1=pe6[:, j, :, None].to_broadcast(
                                [128, heads, HID]),
                            op=OP.mult)
                        msgf = msg[:].rearrange("p a b -> p (a b)")
                        for j0 in range(0, heads * HID, 512):
                            nc.tensor.matmul(
                                out=agg[:, j0:j0 + 512],
                                lhsT=oh_all[:, c, :],
                                rhs=msgf[:, j0:j0 + 512],
                                start=first, stop=last,
                                skip_group_check=True)
                        if last:
                            epilogue(bb, agg, den)

                def epilogue(bb, agg, den):
                    rec = bpool.tile([128, heads], FP32, tag="rec")
                    # pad dst lanes have denom 0; tiny floor, output discarded
                    nc.vector.tensor_scalar_max(out=rec[:], in0=den[:],
                                                scalar1=1e-20)
                    nc.vector.reciprocal(out=rec[:], in_=rec[:])
                    sc = bpool.tile([128, heads, HID], BF16, tag="sc")
                    nc.vector.tensor_tensor(
                        out=sc[:],
                        in0=agg[:].rearrange("p (a b) -> p a b", a=heads),
                        in1=rec[:, :, None].to_broadcast([128, heads, HID]),
                        op=OP.mult)
                    out_ps = agg[:, 0:128]  # agg region is dead after sc
                    for hh in range(heads):
                        tr = psT.tile([128, 128], BF16, tag="tr",
                                      name=f"tr{li}_{bb}_{hh}")
                        nc.tensor.transpose(out=tr[:], in_=sc[:, hh, :],
                                            identity=ident_bf[:])
                        sct = bpool.tile([128, 128], BF16, tag="sct")
                        nc.scalar.activation(out=sct[:], in_=tr[:],
                                             func=AF.Copy)
                        nc.tensor.matmul(
                            out=out_ps,
                            lhsT=Wl_bf[:, li, hh * HID:(hh + 1) * HID],
                            rhs=sct[:],
                            start=(hh == 0), stop=(hh == heads - 1),
                            skip_group_check=True)
                    z1 = bpool.tile([128, 128], FP32, tag="z1")
                    nc.scalar.activation(out=z1[:], in_=out_ps,
                                         func=AF.Identity,
                                         bias=t_aff[:, li:li + 1],
                                         scale=s_aff[:, li:li + 1])
                    z2 = bpool.tile([128, 128], FP32, tag="z2")
                    nc.vector.tensor_scalar_mul(
                        out=z2[:], in0=hprev[:, bb * 128:(bb + 1) * 128],
                        scalar1=ALPHA)
                    nc.vector.tensor_tensor(out=z1[:], in0=z1[:], in1=z2[:],
                                            op=OP.add)
                    _elu(nc, bpool, hnew[:, bb * 128:(bb + 1) * 128], z1[:],
                         (128, 128), "n")

                S = len(smetas)
                for s in range(S + 1):
                    if s < S:
                        stageA(s)
                    if s >= 1:
                        stageB(s - 1)

            # ---------------- classifier ----------------
            hfin = h_own[layers % 2]
            zc = wpool.tile([qd, npc_pad], FP32, tag="zw2")
            c1ps = psA.tile([128, 1024], FP32, tag="agg", name="c1ps")
            c1tl = psD.tile([128, 256], FP32, tag="den", name="c1tl")
            piecesC = [(0, 512, c1ps[:qd, 0:512]), (512, 1024, c1ps[:qd, 512:1024]),
                       (1024, npc_pad, c1tl[:qd, 0:npc_pad - 1024])]
            for (j0, j1, ps) in piecesC:
                nc.tensor.matmul(out=ps, lhsT=W1_sb[:],
                                 rhs=hfin[:, j0:j1], start=True, stop=True,
                                 skip_group_check=True)
                nc.scalar.activation(out=zc[:, j0:j1], in_=ps,
                                     func=AF.Identity,
                                     bias=b1_sb[:, :1], scale=1.0)
            hidsb = wpool.tile([qd, npc_pad], FP32, tag="hidsb")
            _elu(nc, wpool, hidsb[:], zc[:], (qd, npc_pad), "w")
            osb = wpool.tile([CLS, npc_pad], FP32, tag="osb")
            c2ps = psA.tile([128, 1024], FP32, tag="agg", name="c2ps")
            c2tl = psD.tile([128, 256], FP32, tag="den", name="c2tl")
            piecesO = [(0, 512, c2ps[:CLS, 0:512]), (512, 1024, c2ps[:CLS, 512:1024]),
                       (1024, npc_pad, c2tl[:CLS, 0:npc_pad - 1024])]
            for (j0, j1, ps) in piecesO:
                nc.tensor.matmul(out=ps, lhsT=W2_sb[:],
                                 rhs=hidsb[:, j0:j1], start=True, stop=True,
                                 skip_group_check=True)
                nc.scalar.activation(out=osb[:, j0:j1], in_=ps,
                                     func=AF.Identity,
                                     bias=b2_sb[:, :1], scale=1.0)
            nc.sync.dma_start(out=out_dram[:, :], in_=osb[:])

    return nc


_LAST_EXEC_NS = None


def _run(inputs, trace=False):
    global _LAST_EXEC_NS
    from concourse.bass_utils import run_bass_kernel_spmd

    cfg, shared, per_core = preprocess(**inputs)
    nc = bacc.Bacc("TRN2", target_bir_lowering=False, debug=False,
                   num_devices=M)
    build(nc, cfg)
    nc.compile()

    in_maps = []
    for k in range(M):
        m = dict(shared)
        m.update(per_core[k])
        in_maps.append({k2: np.ascontiguousarray(v) for k2, v in m.items()})

    res = run_bass_kernel_spmd(nc, in_maps, list(range(M)), trace=trace)
    _LAST_EXEC_NS = res.exec_time_ns

    out = np.zeros((N, CLS), np.float32)
    for k in range(M):
        o = res.results[k]["out"]  # [CLS, NPC_PAD]
        out[k * NPC:(k + 1) * NPC] = o[:CLS, :NPC].T
    return out


def kernel(**inputs):
    return _run(inputs, trace=False)


# revision 15
# speedup vs baseline: 1.0189x; 1.0189x over previous
"""DeepGAT (4-layer GAT + BN + residual + MLP head) on 8 Trainium2 cores.

Sharding: nodes are dst-partitioned across the 8 cores (1250 nodes/core).
Edges are routed on the host to the core owning their dst node and sorted by
dst. Weights are replicated.

Key idea vs the straightforward scheme: aggregate in INPUT space. Since
xl = h @ W is linear, sum_e att_e * xl[src_e] = (sum_e att_e * h[src_e]) @ W,
so each edge only needs the 128-dim h row (bf16, 512B padded) instead of the
1024-dim projected row. Per layer each core:
  1. transposes its own h block-wise to node-major, appends alpha_src
     (h @ ws), writes [h | a_src | pad] rows to a bounce buffer,
  2. AllGathers the bounce -> full node-major gather table [10240, 256] bf16,
  3. per dst block: dma_gathers src rows, computes pe = exp(lrelu(as+ad))
     with alpha_dst expanded edge-wise by a one-hot matmul (no second
     gather), aggregates agg[dst] += pe * h_src and den[dst] += pe via
     one-hot matmuls into PSUM,
  4. epilogue: normalizes by den per (dst, head), transposes per head and
     projects with W_head (accumulating the head mean), then affine (BN) +
     residual + elu.
The one-hot matrices oh [e,d] / ohT [d,e] are layer-invariant and stay
resident in SBUF. Softmax normalization is applied AFTER aggregation
(mathematically identical, single edge pass).
"""

import numpy as np

import concourse.bass as bass
import concourse.bacc as bacc
import concourse.mybir as mybir
from concourse.tile import TileContext
from concourse.tile_rust import add_dep_helper

FP32 = mybir.dt.float32
BF16 = mybir.dt.bfloat16
I16 = mybir.dt.int16
AF = mybir.ActivationFunctionType
OP = mybir.AluOpType

# problem constants (hardcoded per harness contract)
ALPHA = 0.1
BN_EPS = 1e-5
NEG_SLOPE = 0.2
NEG_BIG = -30000.0
HID = 128  # partition width; fixed
ROWE = 256  # gather row elements (bf16): [h(128) | a_src(8) | pad] = 512B
SPLIT_CHUNKS = 6  # target chunks per dma_gather


def _set_dims(n=10000, e=160000, in_dim=512, heads=8, layers=4, cls=2, cores=8):
    """Set problem dims as module globals (parametrized for sim tests)."""
    g = globals()
    g["N"], g["E"], g["IN"], g["H"], g["L"], g["CLS"], g["M"] = (
        n, e, in_dim, heads, layers, cls, cores)
    g["NPC"] = n // cores
    g["NPC_PAD"] = -(-g["NPC"] // 128) * 128
    g["NBLK"] = g["NPC_PAD"] // 128
    g["NROWS"] = cores * g["NPC_PAD"]


_set_dims()


class Cfg:
    """Static schedule computed from the actual edge data."""

    def __init__(self, chunks_per_block):
        self.chunks_per_block = list(chunks_per_block)
        self.CH = sum(self.chunks_per_block)
        self.TOTE = 128 * self.CH
        self.CHP = -(-self.CH // 128) * 128


def _pack_idx16(idx, pad_to=None):
    """Pack int16 indices for dma_gather: idx i at [i%16, i//16], replicated
    to 128 partitions."""
    idx = np.asarray(idx, np.int64)
    n = len(idx)
    if pad_to is not None:
        assert pad_to >= n
        idx = np.concatenate([idx, np.zeros(pad_to - n, np.int64)])
        n = pad_to
    assert n % 16 == 0
    a = idx.astype(np.int16).reshape(n // 16, 16).T  # [16, n//16]
    return np.tile(a, (8, 1)).copy()  # [128, n//16]


def preprocess(x, edge_index, Wp, bp, Wl, att_src, att_dst, bl, gamma, beta,
               W1, b1, W2, b2):
    """Host-side: edge routing/sorting per core + weight folding."""
    x = np.asarray(x, np.float32)
    src = np.concatenate([np.asarray(edge_index[0]), np.arange(N)]).astype(np.int64)
    dst = np.concatenate([np.asarray(edge_index[1]), np.arange(N)]).astype(np.int64)

    per_core = []
    for k in range(M):
        m = (dst // NPC) == k
        s_k, d_k = src[m], dst[m] - k * NPC
        order = np.argsort(d_k, kind="stable")
        per_core.append((s_k[order], d_k[order]))

    counts = np.zeros((M, NBLK), np.int64)
    for k in range(M):
        _, d_k = per_core[k]
        b = d_k // 128
        for bb in range(NBLK):
            counts[k, bb] = int((b == bb).sum())
    chunks_per_block = [max(1, int(np.ceil(counts[:, bb].max() / 128)))
                        for bb in range(NBLK)]
    cfg = Cfg(chunks_per_block)

    per_core_inputs = []
    for k in range(M):
        s_k, d_k = per_core[k]
        b_k = d_k // 128
        srcrow = np.zeros(cfg.TOTE, np.int64)
        dstloc = np.zeros(cfg.TOTE, np.int64)
        padb = np.full(cfg.TOTE, NEG_BIG, np.float32)
        off = 0
        for bb in range(NBLK):
            sel = b_k == bb
            cnt = int(sel.sum())
            cap = 128 * cfg.chunks_per_block[bb]
            assert cnt <= cap, (k, bb, cnt, cap)
            s_sel = s_k[sel]
            # global node id -> padded table row id
            srcrow[off:off + cnt] = (s_sel // NPC) * NPC_PAD + (s_sel % NPC)
            dstloc[off:off + cnt] = d_k[sel] - 128 * bb
            padb[off:off + cnt] = 0.0
            off += cap
        assert off == cfg.TOTE

        dloc_cols = dstloc.reshape(cfg.CH, 128)  # [CH, 128]

        xT_own = np.zeros((IN, NPC_PAD), np.float32)
        xT_own[:, :NPC] = x[k * NPC:(k + 1) * NPC].T

        per_core_inputs.append({
            "srcidx": _pack_idx16(srcrow),
            "dstloc": dloc_cols.T.astype(np.float32).copy(),
            "padbias": padb.reshape(cfg.CH, 128).T.copy(),
            "xT_own": xT_own,
        })

    # weight folding
    Wl = np.asarray(Wl, np.float32)          # [L, HID, H*HID]
    a_s = np.asarray(att_src, np.float32)    # [L, H, HID]
    a_d = np.asarray(att_dst, np.float32)
    wsd = np.zeros((HID, L * 2 * H), np.float32)
    for i in range(L):
        w3 = Wl[i].reshape(HID, H, HID)
        wsd[:, i * 2 * H:i * 2 * H + H] = np.einsum("khc,hc->kh", w3, a_s[i])
        wsd[:, i * 2 * H + H:(i + 1) * 2 * H] = np.einsum("khc,hc->kh", w3, a_d[i])

    bn_inv = 1.0 / np.sqrt(1.0 + BN_EPS)
    gamma = np.asarray(gamma, np.float32)
    beta = np.asarray(beta, np.float32)
    bl = np.asarray(bl, np.float32)
    # h = elu((1-a)*(gamma*bn_inv*(mean+bl)+beta) + a*prev); fold 1/H into s.
    s_aff = ((1.0 - ALPHA) * gamma * bn_inv / H).T.copy()            # [HID, L]
    t_aff = ((1.0 - ALPHA) * (gamma * bn_inv * bl + beta)).T.copy()  # [HID, L]

    iota_sq = np.broadcast_to(np.arange(128, dtype=np.float32), (128, 128)).copy()
    ident = np.eye(128, dtype=np.float32)

    shared = {
        "Wp": np.asarray(Wp, np.float32),
        "bp": np.asarray(bp, np.float32)[:, None],
        "Wl3": Wl,  # [L, HID, H*HID] fp32, bf16-truncated on device
        "wsd": wsd,
        "s_aff": s_aff, "t_aff": t_aff,
        "W1": np.asarray(W1, np.float32),
        "b1": np.asarray(b1, np.float32)[:, None],
        "W2": np.asarray(W2, np.float32),
        "b2": np.asarray(b2, np.float32)[:, None],
        "iota_sq": iota_sq, "ident": ident,
    }
    return cfg, shared, per_core_inputs


def _elu(nc, p, out_ap, z_ap, shape, tg):
    """out = elu(z) = relu(z) + exp(min(z,0)) - 1, z in SBUF f32."""
    P, F = shape
    mn = p.tile([P, F], FP32, tag=f"elu_mn_{tg}")
    ex = p.tile([P, F], FP32, tag=f"elu_ex_{tg}")
    rl = p.tile([P, F], FP32, tag=f"elu_rl_{tg}")
    nc.vector.tensor_scalar_min(out=mn[:], in0=z_ap, scalar1=0.0)
    nc.scalar.activation(out=ex[:], in_=mn[:], func=AF.Exp)
    nc.vector.tensor_scalar_max(out=rl[:], in0=z_ap, scalar1=0.0)
    nc.vector.tensor_tensor(out=rl[:], in0=rl[:], in1=ex[:], op=OP.add)
    nc.vector.tensor_scalar_sub(out=out_ap, in0=rl[:], scalar1=1.0)


def build(nc, cfg):
    """Emit the SPMD program (dims from module globals)."""
    n, npc, npc_pad = N, NPC, NPC_PAD
    in_dim, layers, heads, cores = IN, L, H, M
    nblk = NBLK
    qd = HID // 2
    CH, CHP = cfg.CH, cfg.CHP

    # ---------------- I/O ----------------
    srcidx = nc.dram_tensor("srcidx", [128, cfg.TOTE // 16], I16, kind="ExternalInput")
    dstloc_in = nc.dram_tensor("dstloc", [128, CH], FP32, kind="ExternalInput")
    padbias_in = nc.dram_tensor("padbias", [128, CH], FP32, kind="ExternalInput")
    xT_own_in = nc.dram_tensor("xT_own", [in_dim, npc_pad], FP32, kind="ExternalInput")
    Wp_in = nc.dram_tensor("Wp", [in_dim, HID], FP32, kind="ExternalInput")
    bp_in = nc.dram_tensor("bp", [HID, 1], FP32, kind="ExternalInput")
    Wl3_in = nc.dram_tensor("Wl3", [layers, HID, heads * HID], FP32,
                            kind="ExternalInput")
    wsd_in = nc.dram_tensor("wsd", [HID, layers * 2 * heads], FP32,
                            kind="ExternalInput")
    s_aff_in = nc.dram_tensor("s_aff", [HID, layers], FP32, kind="ExternalInput")
    t_aff_in = nc.dram_tensor("t_aff", [HID, layers], FP32, kind="ExternalInput")
    W1_in = nc.dram_tensor("W1", [HID, qd], FP32, kind="ExternalInput")
    b1_in = nc.dram_tensor("b1", [qd, 1], FP32, kind="ExternalInput")
    W2_in = nc.dram_tensor("W2", [qd, CLS], FP32, kind="ExternalInput")
    b2_in = nc.dram_tensor("b2", [CLS, 1], FP32, kind="ExternalInput")
    iota_in = nc.dram_tensor("iota_sq", [128, 128], FP32, kind="ExternalInput")
    ident_in = nc.dram_tensor("ident", [128, 128], FP32, kind="ExternalInput")
    out_dram = nc.dram_tensor("out", [CLS, npc_pad], FP32, kind="ExternalOutput")

    table = nc.dram_tensor("h_table", [NROWS, ROWE], BF16,
                           addr_space="Shared" if cores > 4 else "Local")

    with TileContext(nc) as tc:
        with (
            tc.tile_pool(name="const", bufs=1) as cpool,
            tc.tile_pool(name="hbuf", bufs=1) as hpool,
            tc.tile_pool(name="row", bufs=2) as rpool,
            tc.tile_pool(name="gath", bufs=4) as gpool,
            tc.tile_pool(name="edge", bufs=3) as epool,
            tc.tile_pool(name="blk", bufs=2) as bpool,
            tc.tile_pool(name="wide", bufs=1) as wpool,
            tc.tile_pool(name="dram", bufs=1, space="DRAM") as dpool,
            tc.tile_pool(name="psA", bufs=2, space="PSUM") as psA,
            tc.tile_pool(name="psD", bufs=1, space="PSUM") as psD,
            tc.tile_pool(name="psE", bufs=1, space="PSUM") as psE,
            tc.tile_pool(name="psT", bufs=2, space="PSUM") as psT,
        ):
            # dma_gather allocates a register per distinct count; cache them
            _regs = {}

            def nreg(v):
                if v not in _regs:
                    _regs[v] = nc.gpsimd.to_reg(v)
                return _regs[v]

            dma_sem = nc.alloc_semaphore("swdge_dma")

            # ---------------- resident constants / state ----------------
            iota_bf = cpool.tile([128, 128], BF16)
            nc.gpsimd.dma_start(out=iota_bf[:], in_=iota_in[:, :])
            ident_f = cpool.tile([128, 128], FP32)
            nc.sync.dma_start(out=ident_f[:], in_=ident_in[:, :])
            ident_bf = cpool.tile([128, 128], BF16)
            nc.gpsimd.dma_start(out=ident_bf[:], in_=ident_in[:, :])
            dstloc_f = cpool.tile([128, CH], FP32)
            nc.sync.dma_start(out=dstloc_f[:], in_=dstloc_in[:, :])
            srcidx_sb = cpool.tile([128, cfg.TOTE // 16], I16)
            nc.sync.dma_start(out=srcidx_sb[:], in_=srcidx[:, :])
            padbias = cpool.tile([128, CH], FP32)
            nc.sync.dma_start(out=padbias[:], in_=padbias_in[:, :])
            wsd_sb = cpool.tile([128, layers * 2 * heads], FP32)
            nc.sync.dma_start(out=wsd_sb[:], in_=wsd_in[:, :])
            Wl_bf = cpool.tile([128, layers, heads * HID], BF16)
            for li in range(layers):
                nc.gpsimd.dma_start(out=Wl_bf[:, li, :], in_=Wl3_in[li, :, :])
            s_aff = cpool.tile([128, layers], FP32)
            nc.sync.dma_start(out=s_aff[:], in_=s_aff_in[:, :])
            t_aff = cpool.tile([128, layers], FP32)
            nc.sync.dma_start(out=t_aff[:], in_=t_aff_in[:, :])
            W1_sb = cpool.tile([128, qd], FP32)
            nc.sync.dma_start(out=W1_sb[:], in_=W1_in[:, :])
            b1_sb = cpool.tile([qd, 1], FP32)
            nc.sync.dma_start(out=b1_sb[:], in_=b1_in[:, :])
            W2_sb = cpool.tile([qd, CLS], FP32)
            nc.sync.dma_start(out=W2_sb[:], in_=W2_in[:, :])
            b2_sb = cpool.tile([CLS, 1], FP32)
            nc.sync.dma_start(out=b2_sb[:], in_=b2_in[:, :])
            bp_sb = cpool.tile([HID, 1], FP32)
            nc.sync.dma_start(out=bp_sb[:], in_=bp_in[:, :])

            h_own = [hpool.tile([128, npc_pad], FP32, tag=f"h_own{i}",
                                name=f"h_own{i}")
                     for i in range(2)]
            ad_all = hpool.tile([128, nblk * heads], BF16, tag="ad_all")

            # one-hot matrices, layer-invariant, SBUF-resident
            oh_all = cpool.tile([128, CH, 128], BF16)
            ohT_all = cpool.tile([128, CH, 128], BF16)
            for c in range(CH):
                nc.vector.tensor_scalar(
                    out=oh_all[:, c, :], in0=iota_bf[:],
                    scalar1=dstloc_f[:, c:c + 1], scalar2=None,
                    op0=OP.is_equal)
                ohT_ps = psT.tile([128, 128], BF16, tag="tr", name=f"ohT{c}")
                nc.tensor.transpose(out=ohT_ps[:], in_=oh_all[:, c, :],
                                    identity=ident_bf[:])
                nc.scalar.activation(out=ohT_all[:, c, :], in_=ohT_ps[:],
                                     func=AF.Copy)

            kchunks = in_dim // 128

            # ------- h0 = elu(x @ Wp + bp), own nodes only ----
            with tc.tile_pool(name="x0", bufs=2) as x0pool:
                Wp_sb = cpool.tile([128, kchunks, HID], FP32)
                for kc in range(kchunks):
                    nc.sync.dma_start(out=Wp_sb[:, kc, :],
                                      in_=Wp_in[kc * 128:(kc + 1) * 128, :])
                z0 = wpool.tile([128, npc_pad], FP32, tag="zw")
                h0a = psA.tile([128, 1024], FP32, tag="agg", name="h0a")
                h0b = psD.tile([128, 256], FP32, tag="den", name="h0b")
                pieces0 = [(0, 512, h0a[:, 0:512]), (512, 1024, h0a[:, 512:1024]),
                           (1024, npc_pad, h0b[:, 0:npc_pad - 1024])]
                for kc in range(kchunks):
                    xt = x0pool.tile([128, npc_pad], FP32, tag="xT",
                                     name=f"xT{kc}")
                    nc.sync.dma_start(out=xt[:],
                                      in_=xT_own_in[kc * 128:(kc + 1) * 128, :])
                    for (j0, j1, ps) in pieces0:
                        nc.tensor.matmul(out=ps,
                                         lhsT=Wp_sb[:, kc, :],
                                         rhs=xt[:, j0:j1],
                                         start=(kc == 0),
                                         stop=(kc == kchunks - 1),
                                         skip_group_check=True)
                for (j0, j1, ps) in pieces0:
                    nc.scalar.activation(out=z0[:, j0:j1], in_=ps,
                                         func=AF.Identity,
                                         bias=bp_sb[:, :1], scale=1.0)
                _elu(nc, wpool, h_own[0][:], z0[:], (128, npc_pad), "w")

            # ---------------- layers ----------------
            for li in range(layers):
                hprev = h_own[li % 2]
                hnew = h_own[(li + 1) % 2]

                # --- own-table build: node-major [h | a_src] rows ---
                bounce = dpool.tile([npc_pad, ROWE], BF16, tag="bounce")
                for bb in range(nblk):
                    hb_bf = rpool.tile([128, 128], BF16, tag="hbf")
                    nc.vector.tensor_copy(
                        out=hb_bf[:], in_=hprev[:, bb * 128:(bb + 1) * 128])
                    al_ps = psE.tile([128, 2 * heads], FP32, tag="ade",
                                     name=f"al{li}_{bb}")
                    nc.tensor.matmul(out=al_ps[:],
                                     lhsT=hprev[:, bb * 128:(bb + 1) * 128],
                                     rhs=wsd_sb[:, li * 2 * heads:
                                                (li + 1) * 2 * heads],
                                     start=True, stop=True,
                                     skip_group_check=True)
                    t_ps = psT.tile([128, 128], BF16, tag="tr",
                                    name=f"tps{li}_{bb}")
                    nc.tensor.transpose(out=t_ps[:], in_=hb_bf[:],
                                        identity=ident_bf[:])
                    row_sb = rpool.tile([128, ROWE], BF16, tag="row")
                    nc.scalar.activation(out=row_sb[:, 0:128], in_=t_ps[:],
                                         func=AF.Copy)
                    nc.vector.tensor_copy(out=row_sb[:, 128:128 + heads],
                                          in_=al_ps[:, 0:heads])
                    nc.vector.tensor_copy(
                        out=ad_all[:, bb * heads:(bb + 1) * heads],
                        in_=al_ps[:, heads:2 * heads])
                    nc.sync.dma_start(
                        out=bounce[bb * 128:(bb + 1) * 128, :], in_=row_sb[:])

                # --- AllGather: bounce -> full gather table ---
                cc = nc.gpsimd.collective_compute(
                    "AllGather", OP.bypass,
                    replica_groups=[list(range(cores))],
                    ins=[bounce[:]], outs=[table[:, :]],
                )

                # --- edge phase: per-split batched attention, pipelined ---
                smetas = []
                nwave = 0
                off = 0
                for bb in range(nblk):
                    cb = cfg.chunks_per_block[bb]
                    nsp = max(1, -(-cb // SPLIT_CHUNKS))
                    base = cb // nsp
                    rem = cb % nsp
                    sizes = [base + (1 if i < rem else 0) for i in range(nsp)]
                    lo = 0
                    for sz in sizes:
                        g = gpool.tile([128, sz, ROWE], BF16, tag="gt")
                        # prep descriptors early (data-independent); the
                        # trigger carries the table-ready (AllGather) dep
                        nc.gpsimd.dma_gather(
                            out_ap=g[:], in_ap=table[:, :],
                            idxs_ap=srcidx_sb[:, (off + lo) * 8:
                                              (off + lo + sz) * 8],
                            num_idxs=128 * sz, num_idxs_reg=nreg(128 * sz),
                            elem_size=ROWE, single_packet=128 * sz <= 1024,
                            prepare_only=True, sem=dma_sem)
                        nwave += 1
                        if nwave == 4:
                            trig = nc.gpsimd.trigger_dma(count=None)
                            add_dep_helper(trig.ins, cc.ins, True,
                                           "tbl->trigger")
                            nwave = 0
                        smetas.append((off + lo, bb, g, sz, lo, cb))
                        lo += sz
                    off += cb
                if nwave:
                    trig = nc.gpsimd.trigger_dma(count=None)
                    add_dep_helper(trig.ins, cc.ins, True, "tbl->trigger")
                    nwave = 0

                state = {}
                pend = {}

                def stageA(s):
                    c0, bb, gt, sz, lo, cb = smetas[s]
                    ade6 = psE.tile([128, sz, heads], FP32, tag="ade",
                                    name=f"ade{li}_{c0}")
                    for j in range(sz):
                        nc.tensor.matmul(
                            out=ade6[:, j, :], lhsT=ohT_all[:, c0 + j, :],
                            rhs=ad_all[:, bb * heads:(bb + 1) * heads],
                            start=True, stop=True, skip_group_check=True)
                    # sv = a_src[gathered] + a_dst[one-hot] + padbias
                    sv6 = epool.tile([128, sz, heads], FP32, tag="sv")
                    nc.vector.tensor_tensor(
                        out=sv6[:], in0=gt[:, :, 128:128 + heads],
                        in1=ade6[:], op=OP.add)
                    nc.vector.tensor_tensor(
                        out=sv6[:], in0=sv6[:],
                        in1=padbias[:, c0:c0 + sz, None].to_broadcast(
                            [128, sz, heads]),
                        op=OP.add)
                    # exp(lrelu(x)) = max(exp(x), exp(0.2*x)), exp monotone;
                    # pads carry -3e4 so both exps are 0 there
                    e1 = epool.tile([128, sz, heads], FP32, tag="e1")
                    nc.scalar.activation(
                        out=e1[:].rearrange("p a b -> p (a b)"),
                        in_=sv6[:].rearrange("p a b -> p (a b)"), func=AF.Exp)
                    e2 = epool.tile([128, sz, heads], FP32, tag="e2")
                    nc.scalar.activation(
                        out=e2[:].rearrange("p a b -> p (a b)"),
                        in_=sv6[:].rearrange("p a b -> p (a b)"), func=AF.Exp,
                        scale=NEG_SLOPE)
                    pe6 = epool.tile([128, sz, heads], BF16, tag="pe6")
                    nc.vector.tensor_tensor(out=pe6[:], in0=e1[:], in1=e2[:],
                                            op=OP.max)
                    pend[s] = pe6

                def stageB(s):
                    c0, bb, gt, sz, lo, cb = smetas[s]
                    pe6 = pend.pop(s)
                    for j in range(sz):
                        c = c0 + j
                        first, last = lo + j == 0, lo + j == cb - 1
                        if first:
                            state[bb] = (
                                psA.tile([128, heads * HID], FP32, tag="agg",
                                         name=f"agg{li}_{bb}"),
                                psD.tile([128, heads], FP32, tag="den",
                                         name=f"den{li}_{bb}"))
                        agg, den = state[bb]
                        nc.tensor.matmul(
                            out=den[:], lhsT=oh_all[:, c, :],
                            rhs=pe6[:, j, :],
                            start=first, stop=last, skip_group_check=True)
                        msg = epool.tile([128, heads, HID], BF16, tag="msg")
                        nc.vector.tensor_tensor(
                            out=msg[:],
                            in0=gt[:, j:j + 1, 0:HID].to_broadcast(
                                [128, heads, HID]),
                            in1=pe6[:, j, :, None].to_broadcast(
                                [128, heads, HID]),
                            op=OP.mult)
                        msgf = msg[:].rearrange("p a b -> p (a b)")
                        for j0 in range(0, heads * HID, 512):
                            nc.tensor.matmul(
                                out=agg[:, j0:j0 + 512],
                                lhsT=oh_all[:, c, :],
                                rhs=msgf[:, j0:j0 + 512],
                                start=first, stop=last,
                                skip_group_check=True)
                        if last:
                            epilogue(bb, agg, den)

                def epilogue(bb, agg, den):
                    rec = bpool.tile([128, heads], FP32, tag="rec")
                    # pad dst lanes have denom 0; tiny floor, output discarded
                    nc.vector.tensor_scalar_max(out=rec[:], in0=den[:],
                                                scalar1=1e-20)
                    nc.vector.reciprocal(out=rec[:], in_=rec[:])
                    sc = bpool.tile([128, heads, HID], BF16, tag="sc")
                    nc.vector.tensor_tensor(
                        out=sc[:],
                        in0=agg[:].rearrange("p (a b) -> p a b", a=heads),
                        in1=rec[:, :, None].to_broadcast([128, heads, HID]),
                        op=OP.mult)
                    out_ps = agg[:, 0:128]  # agg region is dead after sc
                    for hh in range(heads):
                        tr = psT.tile([128, 128], BF16, tag="tr",
                                      name=f"tr{li}_{bb}_{hh}")
                        nc.tensor.transpose(out=tr[:], in_=sc[:, hh, :],
                                            identity=ident_bf[:])
                        sct = bpool.tile([128, 128], BF16, tag="sct")
                        nc.scalar.activation(out=sct[:], in_=tr[:],
                                             func=AF.Copy)
                        nc.tensor.matmul(
                            out=out_ps,
                            lhsT=Wl_bf[:, li, hh * HID:(hh + 1) * HID],
                            rhs=sct[:],
                            start=(hh == 0), stop=(hh == heads - 1),
                            skip_group_check=True)
                    z1 = bpool.tile([128, 128], FP32, tag="z1")
                    nc.scalar.activation(out=z1[:], in_=out_ps,
                                         func=AF.Identity,
                                         bias=t_aff[:, li:li + 1],
                                         scale=s_aff[:, li:li + 1])
                    z2 = bpool.tile([128, 128], FP32, tag="z2")
                    nc.vector.tensor_scalar_mul(
                        out=z2[:], in0=hprev[:, bb * 128:(bb + 1) * 128],
                        scalar1=ALPHA)
                    nc.vector.tensor_tensor(out=z1[:], in0=z1[:], in1=z2[:],
                                            op=OP.add)
                    _elu(nc, bpool, hnew[:, bb * 128:(bb + 1) * 128], z1[:],
                         (128, 128), "n")

                S = len(smetas)
                for s in range(S + 1):
                    if s < S:
                        stageA(s)
                    if s >= 1:
                        stageB(s - 1)

            # ---------------- classifier ----------------
            hfin = h_own[layers % 2]
            zc = wpool.tile([qd, npc_pad], FP32, tag="zw2")
            c1ps = psA.tile([128, 1024], FP32, tag="agg", name="c1ps")
            c1tl = psD.tile([128, 256], FP32, tag="den", name="c1tl")
            piecesC = [(0, 512, c1ps[:qd, 0:512]), (512, 1024, c1ps[:qd, 512:1024]),
                       (1024, npc_pad, c1tl[:qd, 0:npc_pad - 1024])]
            for (j0, j1, ps) in piecesC:
                nc.tensor.matmul(out=ps, lhsT=W1_sb[:],
                                 rhs=hfin[:, j0:j1], start=True, stop=True,
                                 skip_group_check=True)
                nc.scalar.activation(out=zc[:, j0:j1], in_=ps,
                                     func=AF.Identity,
                                     bias=b1_sb[:, :1], scale=1.0)
            hidsb = wpool.tile([qd, npc_pad], FP32, tag="hidsb")
            _elu(nc, wpool, hidsb[:], zc[:], (qd, npc_pad), "w")
            osb = wpool.tile([CLS, npc_pad], FP32, tag="osb")
            c2ps = psA.tile([128, 1024], FP32, tag="agg", name="c2ps")
            c2tl = psD.tile([128, 256], FP32, tag="den", name="c2tl")
            piecesO = [(0, 512, c2ps[:CLS, 0:512]), (512, 1024, c2ps[:CLS, 512:1024]),
                       (1024, npc_pad, c2tl[:CLS, 0:npc_pad - 1024])]
            for (j0, j1, ps) in piecesO:
                nc.tensor.matmul(out=ps, lhsT=W2_sb[:],
                                 rhs=hidsb[:, j0:j1], start=True, stop=True,
                                 skip_group_check=True)
                nc.scalar.activation(out=osb[:, j0:j1], in_=ps,
                                     func=AF.Identity,
                                     bias=b2_sb[:, :1], scale=1.0)
            nc.sync.dma_start(out=out_dram[:, :], in_=osb[:])

    return nc


_LAST_EXEC_NS = None


def _run(inputs, trace=False):
    global _LAST_EXEC_NS
    from concourse.bass_utils import run_bass_kernel_spmd

    cfg, shared, per_core = preprocess(**inputs)
    nc = bacc.Bacc("TRN2", target_bir_lowering=False, debug=False,
                   num_devices=M)
    build(nc, cfg)
    nc.compile()

    in_maps = []
    for k in range(M):
        m = dict(shared)
        m.update(per_core[k])
        in_maps.append({k2: np.ascontiguousarray(v) for k2, v in m.items()})

    res = run_bass_kernel_spmd(nc, in_maps, list(range(M)), trace=trace)
    _LAST_EXEC_NS = res.exec_time_ns

    out = np.zeros((N, CLS), np.float32)
    for k in range(M):
        o = res.results[k]["out"]  # [CLS, NPC_PAD]
        out[k * NPC:(k + 1) * NPC] = o[:CLS, :NPC].T
    return out


def kernel(**inputs):
    return _run(inputs, trace=False)


# revision 22
# speedup vs baseline: 1.1765x; 1.1547x over previous
"""DeepGAT (4-layer GAT + BN + residual + MLP head) on 8 Trainium2 cores.

Sharding: nodes are dst-partitioned across the 8 cores (1250 nodes/core).
Edges are routed on the host to the core owning their dst node and sorted by
dst. Weights are replicated.

Key idea vs the straightforward scheme: aggregate in INPUT space. Since
xl = h @ W is linear, sum_e att_e * xl[src_e] = (sum_e att_e * h[src_e]) @ W,
so each edge only needs the 128-dim h row (bf16, 512B padded) instead of the
1024-dim projected row. Per layer each core:
  1. transposes its own h block-wise to node-major, appends alpha_src
     (h @ ws), writes [h | a_src | pad] rows to a bounce buffer,
  2. AllGathers the bounce -> full node-major gather table [10240, 256] bf16,
  3. per dst block: dma_gathers src rows, computes pe = exp(lrelu(as+ad))
     with alpha_dst expanded edge-wise by a one-hot matmul (no second
     gather), aggregates agg[dst] += pe * h_src and den[dst] += pe via
     one-hot matmuls into PSUM,
  4. epilogue: normalizes by den per (dst, head), transposes per head and
     projects with W_head (accumulating the head mean), then affine (BN) +
     residual + elu.
The one-hot matrices oh [e,d] / ohT [d,e] are layer-invariant and stay
resident in SBUF. Softmax normalization is applied AFTER aggregation
(mathematically identical, single edge pass).
"""

import numpy as np

import concourse.bass as bass
import concourse.bacc as bacc
import concourse.mybir as mybir
from concourse.tile import TileContext
from concourse.tile_rust import add_dep_helper

FP32 = mybir.dt.float32
BF16 = mybir.dt.bfloat16
I16 = mybir.dt.int16
AF = mybir.ActivationFunctionType
OP = mybir.AluOpType

# problem constants (hardcoded per harness contract)
ALPHA = 0.1
BN_EPS = 1e-5
NEG_SLOPE = 0.2
NEG_BIG = -30000.0
HID = 128  # partition width; fixed
ROWE = 256  # gather row elements (bf16): [h(128) | a_src(8) | pad] = 512B
SPLIT_CHUNKS = 6  # target chunks per dma_gather


def _set_dims(n=10000, e=160000, in_dim=512, heads=8, layers=4, cls=2, cores=8):
    """Set problem dims as module globals (parametrized for sim tests)."""
    g = globals()
    g["N"], g["E"], g["IN"], g["H"], g["L"], g["CLS"], g["M"] = (
        n, e, in_dim, heads, layers, cls, cores)
    g["NPC"] = n // cores
    g["NPC_PAD"] = -(-g["NPC"] // 128) * 128
    g["NBLK"] = g["NPC_PAD"] // 128
    g["NROWS"] = cores * g["NPC_PAD"]


_set_dims()


class Cfg:
    """Static schedule computed from the actual edge data."""

    def __init__(self, chunks_per_block):
        self.chunks_per_block = list(chunks_per_block)
        self.CH = sum(self.chunks_per_block)
        self.TOTE = 128 * self.CH
        self.CHP = -(-self.CH // 128) * 128


def _pack_idx16(idx, pad_to=None):
    """Pack int16 indices for dma_gather: idx i at [i%16, i//16], replicated
    to 128 partitions."""
    idx = np.asarray(idx, np.int64)
    n = len(idx)
    if pad_to is not None:
        assert pad_to >= n
        idx = np.concatenate([idx, np.zeros(pad_to - n, np.int64)])
        n = pad_to
    assert n % 16 == 0
    a = idx.astype(np.int16).reshape(n // 16, 16).T  # [16, n//16]
    return np.tile(a, (8, 1)).copy()  # [128, n//16]


def preprocess(x, edge_index, Wp, bp, Wl, att_src, att_dst, bl, gamma, beta,
               W1, b1, W2, b2):
    """Host-side: edge routing/sorting per core + weight folding."""
    x = np.asarray(x, np.float32)
    src = np.concatenate([np.asarray(edge_index[0]), np.arange(N)]).astype(np.int64)
    dst = np.concatenate([np.asarray(edge_index[1]), np.arange(N)]).astype(np.int64)

    per_core = []
    for k in range(M):
        m = (dst // NPC) == k
        s_k, d_k = src[m], dst[m] - k * NPC
        order = np.argsort(d_k, kind="stable")
        per_core.append((s_k[order], d_k[order]))

    counts = np.zeros((M, NBLK), np.int64)
    for k in range(M):
        _, d_k = per_core[k]
        b = d_k // 128
        for bb in range(NBLK):
            counts[k, bb] = int((b == bb).sum())
    chunks_per_block = [max(1, int(np.ceil(counts[:, bb].max() / 128)))
                        for bb in range(NBLK)]
    cfg = Cfg(chunks_per_block)

    per_core_inputs = []
    for k in range(M):
        s_k, d_k = per_core[k]
        b_k = d_k // 128
        srcrow = np.zeros(cfg.TOTE, np.int64)
        dstloc = np.zeros(cfg.TOTE, np.int64)
        padb = np.full(cfg.TOTE, NEG_BIG, np.float32)
        off = 0
        for bb in range(NBLK):
            sel = b_k == bb
            cnt = int(sel.sum())
            cap = 128 * cfg.chunks_per_block[bb]
            assert cnt <= cap, (k, bb, cnt, cap)
            s_sel = s_k[sel]
            # global node id -> padded table row id
            srcrow[off:off + cnt] = (s_sel // NPC) * NPC_PAD + (s_sel % NPC)
            dstloc[off:off + cnt] = d_k[sel] - 128 * bb
            padb[off:off + cnt] = 0.0
            off += cap
        assert off == cfg.TOTE

        dloc_cols = dstloc.reshape(cfg.CH, 128)  # [CH, 128]

        xT_own = np.zeros((IN, NPC_PAD), np.float32)
        xT_own[:, :NPC] = x[k * NPC:(k + 1) * NPC].T

        per_core_inputs.append({
            "srcidx": _pack_idx16(srcrow),
            "dstloc": dloc_cols.T.astype(np.float32).copy(),
            "padbias": padb.reshape(cfg.CH, 128).T.copy(),
            "xT_own": xT_own,
        })

    # weight folding
    Wl = np.asarray(Wl, np.float32)          # [L, HID, H*HID]
    a_s = np.asarray(att_src, np.float32)    # [L, H, HID]
    a_d = np.asarray(att_dst, np.float32)
    wsd = np.zeros((HID, L * 2 * H), np.float32)
    for i in range(L):
        w3 = Wl[i].reshape(HID, H, HID)
        wsd[:, i * 2 * H:i * 2 * H + H] = np.einsum("khc,hc->kh", w3, a_s[i])
        wsd[:, i * 2 * H + H:(i + 1) * 2 * H] = np.einsum("khc,hc->kh", w3, a_d[i])

    bn_inv = 1.0 / np.sqrt(1.0 + BN_EPS)
    gamma = np.asarray(gamma, np.float32)
    beta = np.asarray(beta, np.float32)
    bl = np.asarray(bl, np.float32)
    # h = elu((1-a)*(gamma*bn_inv*(mean+bl)+beta) + a*prev); fold 1/H into s.
    s_aff = ((1.0 - ALPHA) * gamma * bn_inv / H).T.copy()            # [HID, L]
    t_aff = ((1.0 - ALPHA) * (gamma * bn_inv * bl + beta)).T.copy()  # [HID, L]

    iota_sq = np.broadcast_to(np.arange(128, dtype=np.float32), (128, 128)).copy()
    ident = np.eye(128, dtype=np.float32)

    shared = {
        "Wp": np.asarray(Wp, np.float32),
        "bp": np.asarray(bp, np.float32)[:, None],
        "Wl3": Wl,  # [L, HID, H*HID] fp32, bf16-truncated on device
        "wsd": wsd,
        "s_aff": s_aff, "t_aff": t_aff,
        "W1": np.asarray(W1, np.float32),
        "b1": np.asarray(b1, np.float32)[:, None],
        "W2": np.asarray(W2, np.float32),
        "b2": np.asarray(b2, np.float32)[:, None],
        "iota_sq": iota_sq, "ident": ident,
    }
    return cfg, shared, per_core_inputs


def _elu(nc, p, out_ap, z_ap, shape, tg):
    """out = elu(z) = relu(z) + exp(min(z,0)) - 1, z in SBUF f32."""
    P, F = shape
    mn = p.tile([P, F], FP32, tag=f"elu_mn_{tg}")
    ex = p.tile([P, F], FP32, tag=f"elu_ex_{tg}")
    rl = p.tile([P, F], FP32, tag=f"elu_rl_{tg}")
    nc.vector.tensor_scalar_min(out=mn[:], in0=z_ap, scalar1=0.0)
    nc.scalar.activation(out=ex[:], in_=mn[:], func=AF.Exp)
    nc.vector.tensor_scalar_max(out=rl[:], in0=z_ap, scalar1=0.0)
    nc.vector.tensor_tensor(out=rl[:], in0=rl[:], in1=ex[:], op=OP.add)
    nc.vector.tensor_scalar_sub(out=out_ap, in0=rl[:], scalar1=1.0)


def build(nc, cfg):
    """Emit the SPMD program (dims from module globals)."""
    n, npc, npc_pad = N, NPC, NPC_PAD
    in_dim, layers, heads, cores = IN, L, H, M
    nblk = NBLK
    qd = HID // 2
    CH, CHP = cfg.CH, cfg.CHP

    # ---------------- I/O ----------------
    srcidx = nc.dram_tensor("srcidx", [128, cfg.TOTE // 16], I16, kind="ExternalInput")
    dstloc_in = nc.dram_tensor("dstloc", [128, CH], FP32, kind="ExternalInput")
    padbias_in = nc.dram_tensor("padbias", [128, CH], FP32, kind="ExternalInput")
    xT_own_in = nc.dram_tensor("xT_own", [in_dim, npc_pad], FP32, kind="ExternalInput")
    Wp_in = nc.dram_tensor("Wp", [in_dim, HID], FP32, kind="ExternalInput")
    bp_in = nc.dram_tensor("bp", [HID, 1], FP32, kind="ExternalInput")
    Wl3_in = nc.dram_tensor("Wl3", [layers, HID, heads * HID], FP32,
                            kind="ExternalInput")
    wsd_in = nc.dram_tensor("wsd", [HID, layers * 2 * heads], FP32,
                            kind="ExternalInput")
    s_aff_in = nc.dram_tensor("s_aff", [HID, layers], FP32, kind="ExternalInput")
    t_aff_in = nc.dram_tensor("t_aff", [HID, layers], FP32, kind="ExternalInput")
    W1_in = nc.dram_tensor("W1", [HID, qd], FP32, kind="ExternalInput")
    b1_in = nc.dram_tensor("b1", [qd, 1], FP32, kind="ExternalInput")
    W2_in = nc.dram_tensor("W2", [qd, CLS], FP32, kind="ExternalInput")
    b2_in = nc.dram_tensor("b2", [CLS, 1], FP32, kind="ExternalInput")
    iota_in = nc.dram_tensor("iota_sq", [128, 128], FP32, kind="ExternalInput")
    ident_in = nc.dram_tensor("ident", [128, 128], FP32, kind="ExternalInput")
    out_dram = nc.dram_tensor("out", [CLS, npc_pad], FP32, kind="ExternalOutput")

    table = nc.dram_tensor("h_table", [NROWS, ROWE], BF16,
                           addr_space="Shared" if cores > 4 else "Local")

    with TileContext(nc) as tc:
        with (
            tc.tile_pool(name="const", bufs=1) as cpool,
            tc.tile_pool(name="hbuf", bufs=1) as hpool,
            tc.tile_pool(name="row", bufs=2) as rpool,
            tc.tile_pool(name="gath", bufs=4) as gpool,
            tc.tile_pool(name="edge", bufs=3) as epool,
            tc.tile_pool(name="blk", bufs=2) as bpool,
            tc.tile_pool(name="wide", bufs=1) as wpool,
            tc.tile_pool(name="dram", bufs=1, space="DRAM") as dpool,
            tc.tile_pool(name="psA", bufs=2, space="PSUM") as psA,
            tc.tile_pool(name="psD", bufs=1, space="PSUM") as psD,
            tc.tile_pool(name="psE", bufs=1, space="PSUM") as psE,
            tc.tile_pool(name="psT", bufs=2, space="PSUM") as psT,
        ):
            # dma_gather allocates a register per distinct count; cache them
            _regs = {}

            def nreg(v):
                if v not in _regs:
                    _regs[v] = nc.gpsimd.to_reg(v)
                return _regs[v]

            # ---------------- resident constants / state ----------------
            iota_bf = cpool.tile([128, 128], BF16)
            nc.gpsimd.dma_start(out=iota_bf[:], in_=iota_in[:, :])
            ident_f = cpool.tile([128, 128], FP32)
            nc.sync.dma_start(out=ident_f[:], in_=ident_in[:, :])
            ident_bf = cpool.tile([128, 128], BF16)
            nc.gpsimd.dma_start(out=ident_bf[:], in_=ident_in[:, :])
            dstloc_f = cpool.tile([128, CH], FP32)
            nc.sync.dma_start(out=dstloc_f[:], in_=dstloc_in[:, :])
            srcidx_sb = cpool.tile([128, cfg.TOTE // 16], I16)
            nc.sync.dma_start(out=srcidx_sb[:], in_=srcidx[:, :])
            padbias = cpool.tile([128, CH], FP32)
            nc.sync.dma_start(out=padbias[:], in_=padbias_in[:, :])
            wsd_sb = cpool.tile([128, layers * 2 * heads], FP32)
            nc.sync.dma_start(out=wsd_sb[:], in_=wsd_in[:, :])
            Wl_bf = cpool.tile([128, layers, heads * HID], BF16)
            for li in range(layers):
                nc.gpsimd.dma_start(out=Wl_bf[:, li, :], in_=Wl3_in[li, :, :])
            s_aff = cpool.tile([128, layers], FP32)
            nc.sync.dma_start(out=s_aff[:], in_=s_aff_in[:, :])
            t_aff = cpool.tile([128, layers], FP32)
            nc.sync.dma_start(out=t_aff[:], in_=t_aff_in[:, :])
            W1_sb = cpool.tile([128, qd], FP32)
            nc.sync.dma_start(out=W1_sb[:], in_=W1_in[:, :])
            b1_sb = cpool.tile([qd, 1], FP32)
            nc.sync.dma_start(out=b1_sb[:], in_=b1_in[:, :])
            W2_sb = cpool.tile([qd, CLS], FP32)
            nc.sync.dma_start(out=W2_sb[:], in_=W2_in[:, :])
            b2_sb = cpool.tile([CLS, 1], FP32)
            nc.sync.dma_start(out=b2_sb[:], in_=b2_in[:, :])
            bp_sb = cpool.tile([HID, 1], FP32)
            nc.sync.dma_start(out=bp_sb[:], in_=bp_in[:, :])

            h_own = [hpool.tile([128, npc_pad], FP32, tag=f"h_own{i}",
                                name=f"h_own{i}")
                     for i in range(2)]
            ad_all = hpool.tile([128, nblk * heads], BF16, tag="ad_all")

            # one-hot matrices, layer-invariant, SBUF-resident
            oh_all = cpool.tile([128, CH, 128], BF16)
            ohT_all = cpool.tile([128, CH, 128], BF16)
            for c in range(CH):
                nc.vector.tensor_scalar(
                    out=oh_all[:, c, :], in0=iota_bf[:],
                    scalar1=dstloc_f[:, c:c + 1], scalar2=None,
                    op0=OP.is_equal)
                ohT_ps = psT.tile([128, 128], BF16, tag="tr", name=f"ohT{c}")
                nc.tensor.transpose(out=ohT_ps[:], in_=oh_all[:, c, :],
                                    identity=ident_bf[:])
                nc.scalar.activation(out=ohT_all[:, c, :], in_=ohT_ps[:],
                                     func=AF.Copy)

            kchunks = in_dim // 128

            # ------- h0 = elu(x @ Wp + bp), own nodes only ----
            with tc.tile_pool(name="x0", bufs=2) as x0pool:
                Wp_sb = cpool.tile([128, kchunks, HID], FP32)
                for kc in range(kchunks):
                    nc.sync.dma_start(out=Wp_sb[:, kc, :],
                                      in_=Wp_in[kc * 128:(kc + 1) * 128, :])
                z0 = wpool.tile([128, npc_pad], FP32, tag="zw")
                h0a = psA.tile([128, 1024], FP32, tag="agg", name="h0a")
                h0b = psD.tile([128, 256], FP32, tag="den", name="h0b")
                pieces0 = [(0, 512, h0a[:, 0:512]), (512, 1024, h0a[:, 512:1024]),
                           (1024, npc_pad, h0b[:, 0:npc_pad - 1024])]
                for kc in range(kchunks):
                    xt = x0pool.tile([128, npc_pad], FP32, tag="xT",
                                     name=f"xT{kc}")
                    nc.sync.dma_start(out=xt[:],
                                      in_=xT_own_in[kc * 128:(kc + 1) * 128, :])
                    for (j0, j1, ps) in pieces0:
                        nc.tensor.matmul(out=ps,
                                         lhsT=Wp_sb[:, kc, :],
                                         rhs=xt[:, j0:j1],
                                         start=(kc == 0),
                                         stop=(kc == kchunks - 1),
                                         skip_group_check=True)
                for (j0, j1, ps) in pieces0:
                    nc.scalar.activation(out=z0[:, j0:j1], in_=ps,
                                         func=AF.Identity,
                                         bias=bp_sb[:, :1], scale=1.0)
                _elu(nc, wpool, h_own[0][:], z0[:], (128, npc_pad), "w")

            # ---------------- layers ----------------
            for li in range(layers):
                hprev = h_own[li % 2]
                hnew = h_own[(li + 1) % 2]

                # --- own-table build: node-major [h | a_src] rows ---
                bounce = dpool.tile([npc_pad, ROWE], BF16, tag="bounce")
                for bb in range(nblk):
                    hb_bf = rpool.tile([128, 128], BF16, tag="hbf")
                    nc.vector.tensor_copy(
                        out=hb_bf[:], in_=hprev[:, bb * 128:(bb + 1) * 128])
                    al_ps = psE.tile([128, 2 * heads], FP32, tag="ade",
                                     name=f"al{li}_{bb}")
                    nc.tensor.matmul(out=al_ps[:],
                                     lhsT=hprev[:, bb * 128:(bb + 1) * 128],
                                     rhs=wsd_sb[:, li * 2 * heads:
                                                (li + 1) * 2 * heads],
                                     start=True, stop=True,
                                     skip_group_check=True)
                    t_ps = psT.tile([128, 128], BF16, tag="tr",
                                    name=f"tps{li}_{bb}")
                    nc.tensor.transpose(out=t_ps[:], in_=hb_bf[:],
                                        identity=ident_bf[:])
                    row_sb = rpool.tile([128, ROWE], BF16, tag="row")
                    nc.scalar.activation(out=row_sb[:, 0:128], in_=t_ps[:],
                                         func=AF.Copy)
                    nc.vector.tensor_copy(out=row_sb[:, 128:128 + heads],
                                          in_=al_ps[:, 0:heads])
                    nc.vector.tensor_copy(
                        out=ad_all[:, bb * heads:(bb + 1) * heads],
                        in_=al_ps[:, heads:2 * heads])
                    nc.sync.dma_start(
                        out=bounce[bb * 128:(bb + 1) * 128, :], in_=row_sb[:])

                # --- AllGather: bounce -> full gather table ---
                cc = nc.gpsimd.collective_compute(
                    "AllGather", OP.bypass,
                    replica_groups=[list(range(cores))],
                    ins=[bounce[:]], outs=[table[:, :]],
                )

                # --- edge phase: per-split batched attention, pipelined ---
                smetas = []
                off = 0
                for bb in range(nblk):
                    cb = cfg.chunks_per_block[bb]
                    nsp = max(1, -(-cb // SPLIT_CHUNKS))
                    base = cb // nsp
                    rem = cb % nsp
                    sizes = [base + (1 if i < rem else 0) for i in range(nsp)]
                    lo = 0
                    for sz in sizes:
                        g = gpool.tile([128, sz, ROWE], BF16, tag="gt")
                        g_ = nc.gpsimd.dma_gather(
                            out_ap=g[:], in_ap=table[:, :],
                            idxs_ap=srcidx_sb[:, (off + lo) * 8:
                                              (off + lo + sz) * 8],
                            num_idxs=128 * sz, num_idxs_reg=nreg(128 * sz),
                            elem_size=ROWE, single_packet=128 * sz <= 1024)
                        add_dep_helper(g_.ins, cc.ins, True, "tbl->gather")
                        smetas.append((off + lo, bb, g, sz, lo, cb))
                        lo += sz
                    off += cb

                state = {}
                pend = {}

                def stageA(s):
                    c0, bb, gt, sz, lo, cb = smetas[s]
                    ade6 = psE.tile([128, sz, heads], FP32, tag="ade",
                                    name=f"ade{li}_{c0}")
                    for j in range(sz):
                        nc.tensor.matmul(
                            out=ade6[:, j, :], lhsT=ohT_all[:, c0 + j, :],
                            rhs=ad_all[:, bb * heads:(bb + 1) * heads],
                            start=True, stop=True, skip_group_check=True)
                    # sv = a_src[gathered] + a_dst[one-hot] + padbias
                    sv6 = epool.tile([128, sz, heads], FP32, tag="sv")
                    nc.vector.tensor_tensor(
                        out=sv6[:], in0=gt[:, :, 128:128 + heads],
                        in1=ade6[:], op=OP.add)
                    nc.vector.tensor_tensor(
                        out=sv6[:], in0=sv6[:],
                        in1=padbias[:, c0:c0 + sz, None].to_broadcast(
                            [128, sz, heads]),
                        op=OP.add)
                    # exp(lrelu(x)) = max(exp(x), exp(0.2*x)), exp monotone;
                    # pads carry -3e4 so both exps are 0 there
                    e1 = epool.tile([128, sz, heads], FP32, tag="e1")
                    nc.scalar.activation(
                        out=e1[:].rearrange("p a b -> p (a b)"),
                        in_=sv6[:].rearrange("p a b -> p (a b)"), func=AF.Exp)
                    e2 = epool.tile([128, sz, heads], FP32, tag="e2")
                    nc.scalar.activation(
                        out=e2[:].rearrange("p a b -> p (a b)"),
                        in_=sv6[:].rearrange("p a b -> p (a b)"), func=AF.Exp,
                        scale=NEG_SLOPE)
                    pe6 = epool.tile([128, sz, heads], BF16, tag="pe6")
                    nc.vector.tensor_tensor(out=pe6[:], in0=e1[:], in1=e2[:],
                                            op=OP.max)
                    pend[s] = pe6

                def stageB(s):
                    c0, bb, gt, sz, lo, cb = smetas[s]
                    pe6 = pend.pop(s)
                    for j in range(sz):
                        c = c0 + j
                        first, last = lo + j == 0, lo + j == cb - 1
                        if first:
                            state[bb] = (
                                psA.tile([128, heads * HID], FP32, tag="agg",
                                         name=f"agg{li}_{bb}"),
                                psD.tile([128, heads], FP32, tag="den",
                                         name=f"den{li}_{bb}"))
                        agg, den = state[bb]
                        nc.tensor.matmul(
                            out=den[:], lhsT=oh_all[:, c, :],
                            rhs=pe6[:, j, :],
                            start=first, stop=last, skip_group_check=True)
                        msg = epool.tile([128, heads, HID], BF16, tag="msg")
                        hv = 5  # V does heads 0..4, Scalar heads 5..7
                        nc.vector.tensor_tensor(
                            out=msg[:, 0:hv, :],
                            in0=gt[:, j:j + 1, 0:HID].to_broadcast(
                                [128, hv, HID]),
                            in1=pe6[:, j, 0:hv, None].to_broadcast(
                                [128, hv, HID]),
                            op=OP.mult)
                        for hh in range(hv, heads):
                            nc.scalar.activation(
                                out=msg[:, hh, :], in_=gt[:, j, 0:HID],
                                func=AF.Copy,
                                scale=pe6[:, j, hh:hh + 1])
                        msgf = msg[:].rearrange("p a b -> p (a b)")
                        for j0 in range(0, heads * HID, 512):
                            nc.tensor.matmul(
                                out=agg[:, j0:j0 + 512],
                                lhsT=oh_all[:, c, :],
                                rhs=msgf[:, j0:j0 + 512],
                                start=first, stop=last,
                                skip_group_check=True)
                        if last:
                            epilogue(bb, agg, den)

                def epilogue(bb, agg, den):
                    rec = bpool.tile([128, heads], FP32, tag="rec")
                    # pad dst lanes have denom 0; tiny floor, output discarded
                    nc.vector.tensor_scalar_max(out=rec[:], in0=den[:],
                                                scalar1=1e-20)
                    nc.vector.reciprocal(out=rec[:], in_=rec[:])
                    sc = bpool.tile([128, heads, HID], BF16, tag="sc")
                    nc.vector.tensor_tensor(
                        out=sc[:],
                        in0=agg[:].rearrange("p (a b) -> p a b", a=heads),
                        in1=rec[:, :, None].to_broadcast([128, heads, HID]),
                        op=OP.mult)
                    out_ps = agg[:, 0:128]  # agg region is dead after sc
                    for hh in range(heads):
                        tr = psT.tile([128, 128], BF16, tag="tr",
                                      name=f"tr{li}_{bb}_{hh}")
                        nc.tensor.transpose(out=tr[:], in_=sc[:, hh, :],
                                            identity=ident_bf[:])
                        sct = bpool.tile([128, 128], BF16, tag="sct")
                        nc.scalar.activation(out=sct[:], in_=tr[:],
                                             func=AF.Copy)
                        nc.tensor.matmul(
                            out=out_ps,
                            lhsT=Wl_bf[:, li, hh * HID:(hh + 1) * HID],
                            rhs=sct[:],
                            start=(hh == 0), stop=(hh == heads - 1),
                            skip_group_check=True)
                    z1 = bpool.tile([128, 128], FP32, tag="z1")
                    nc.scalar.activation(out=z1[:], in_=out_ps,
                                         func=AF.Identity,
                                         bias=t_aff[:, li:li + 1],
                                         scale=s_aff[:, li:li + 1])
                    z2 = bpool.tile([128, 128], FP32, tag="z2")
                    nc.vector.tensor_scalar_mul(
                        out=z2[:], in0=hprev[:, bb * 128:(bb + 1) * 128],
                        scalar1=ALPHA)
                    nc.vector.tensor_tensor(out=z1[:], in0=z1[:], in1=z2[:],
                                            op=OP.add)
                    _elu(nc, bpool, hnew[:, bb * 128:(bb + 1) * 128], z1[:],
                         (128, 128), "n")

                S = len(smetas)
                for s in range(S + 1):
                    if s < S:
                        stageA(s)
                    if s >= 1:
                        stageB(s - 1)

            # ---------------- classifier ----------------
            hfin = h_own[layers % 2]
            zc = wpool.tile([qd, npc_pad], FP32, tag="zw2")
            c1ps = psA.tile([128, 1024], FP32, tag="agg", name="c1ps")
            c1tl = psD.tile([128, 256], FP32, tag="den", name="c1tl")
            piecesC = [(0, 512, c1ps[:qd, 0:512]), (512, 1024, c1ps[:qd, 512:1024]),
                       (1024, npc_pad, c1tl[:qd, 0:npc_pad - 1024])]
            for (j0, j1, ps) in piecesC:
                nc.tensor.matmul(out=ps, lhsT=W1_sb[:],
                                 rhs=hfin[:, j0:j1], start=True, stop=True,
                                 skip_group_check=True)
                nc.scalar.activation(out=zc[:, j0:j1], in_=ps,
                                     func=AF.Identity,
                                     bias=b1_sb[:, :1], scale=1.0)
            hidsb = wpool.tile([qd, npc_pad], FP32, tag="hidsb")
            _elu(nc, wpool, hidsb[:], zc[:], (qd, npc_pad), "w")
            osb = wpool.tile([CLS, npc_pad], FP32, tag="osb")
            c2ps = psA.tile([128, 1024], FP32, tag="agg", name="c2ps")
            c2tl = psD.tile([128, 256], FP32, tag="den", name="c2tl")
            piecesO = [(0, 512, c2ps[:CLS, 0:512]), (512, 1024, c2ps[:CLS, 512:1024]),
                       (1024, npc_pad, c2tl[:CLS, 0:npc_pad - 1024])]
            for (j0, j1, ps) in piecesO:
                nc.tensor.matmul(out=ps, lhsT=W2_sb[:],
                                 rhs=hidsb[:, j0:j1], start=True, stop=True,
                                 skip_group_check=True)
                nc.scalar.activation(out=osb[:, j0:j1], in_=ps,
                                     func=AF.Identity,
                                     bias=b2_sb[:, :1], scale=1.0)
            nc.sync.dma_start(out=out_dram[:, :], in_=osb[:])

    return nc


_LAST_EXEC_NS = None


def _run(inputs, trace=False):
    global _LAST_EXEC_NS
    from concourse.bass_utils import run_bass_kernel_spmd

    cfg, shared, per_core = preprocess(**inputs)
    nc = bacc.Bacc("TRN2", target_bir_lowering=False, debug=False,
                   num_devices=M)
    build(nc, cfg)
    nc.compile()

    in_maps = []
    for k in range(M):
        m = dict(shared)
        m.update(per_core[k])
        in_maps.append({k2: np.ascontiguousarray(v) for k2, v in m.items()})

    res = run_bass_kernel_spmd(nc, in_maps, list(range(M)), trace=trace)
    _LAST_EXEC_NS = res.exec_time_ns

    out = np.zeros((N, CLS), np.float32)
    for k in range(M):
        o = res.results[k]["out"]  # [CLS, NPC_PAD]
        out[k * NPC:(k + 1) * NPC] = o[:CLS, :NPC].T
    return out


def kernel(**inputs):
    return _run(inputs, trace=False)
